# revision 1
# baseline (speedup 1.0000x reference)
"""Coattention kernel for Trainium2 (Bass/Tile), data-parallel over batch on 8 cores.

Math per batch (all matrices 768x768, N==D==768):
  lo  = L @ Wc^T + bc            io  = I @ Wc^T + bc
  G2  = io^T
  S1  = lo^T... (reference: softmax((lo^T) @ (io^T)) etc.)  -- see reference.py
Implemented with PE matmuls (float32r), PE tile-transposes streamed into the
consumer matmul's stationary operand, ACT exp/scale softmax, DVE evacuations.
"""
import numpy as np

B = 32
D = 768
P = 128
NT = D // P  # 6
N_CORES = 8
NB = B // N_CORES  # batches per core

_cache = {}


def _build(nb, has_bias, repeat=1, hw_loop=0):
    import concourse.bass as bass
    import concourse.mybir as mybir
    import concourse.tile as tile
    from concourse import bacc
    from concourse.masks import make_identity
    from contextlib import ExitStack

    f32 = mybir.dt.float32
    f32r = mybir.dt.float32r
    Exp = mybir.ActivationFunctionType.Exp
    Copy = mybir.ActivationFunctionType.Copy

    nc = bacc.Bacc("TRN2", target_bir_lowering=False, debug=False)

    L_d = nc.dram_tensor("L", [nb, NT, D, P], f32r, kind="ExternalInput").ap()
    I_d = nc.dram_tensor("I", [nb, NT, D, P], f32r, kind="ExternalInput").ap()
    wct_d = nc.dram_tensor("wct", [D, D], f32r, kind="ExternalInput").ap()
    wst_d = nc.dram_tensor("wst", [D, D], f32r, kind="ExternalInput").ap()
    wxt_d = nc.dram_tensor("wxt", [D, D], f32r, kind="ExternalInput").ap()
    bc_d = nc.dram_tensor("bc", [1, D], f32, kind="ExternalInput").ap()
    bs_d = nc.dram_tensor("bs", [1, D], f32, kind="ExternalInput").ap()
    bx_d = nc.dram_tensor("bx", [1, D], f32, kind="ExternalInput").ap()
    out_d = nc.dram_tensor("out", [nb, D, D], f32, kind="ExternalOutput").ap()

    NH = ((0, 512), (512, 768))  # psum-bank-aligned halves of the free dim

    with tile.TileContext(nc) as tc, ExitStack() as ctx:
        sb = ctx.enter_context(tc.tile_pool(name="sb", bufs=1))
        sml = ctx.enter_context(tc.tile_pool(name="sml", bufs=1))
        p_ltt = ctx.enter_context(tc.tile_pool(name="p_ltt", bufs=(2 if has_bias else 4)))
        p_ts = ctx.enter_context(tc.tile_pool(name="p_ts", bufs=(4 if has_bias else 8)))
        p_sm = ctx.enter_context(tc.tile_pool(name="p_sm", bufs=(1 if has_bias else 2)))
        p_tiny = ctx.enter_context(tc.tile_pool(name="p_tiny", bufs=16))
        ps = ctx.enter_context(tc.tile_pool(name="ps", bufs=2, space="PSUM"))
        tps = ctx.enter_context(tc.tile_pool(name="tps", bufs=4, space="PSUM"))

        # --- identities ---
        ident = sml.tile([P, P], f32, tag="ident")
        make_identity(nc, ident[:])
        ident_r = sml.tile([P, P], f32r, tag="identr")
        nc.vector.tensor_copy(ident_r[:], ident[:])

        # --- weights: host pre-rounded to f32r; DMA straight in ---
        w_sb = {}
        for wname, wd in (("wc", wct_d), ("ws", wst_d), ("wx", wxt_d)):
            wt = sb.tile([P, NT, D], f32r, tag="w_" + wname)
            for k in range(NT):
                nc.sync.dma_start(wt[:, k], wd[k * P:(k + 1) * P, :])
            w_sb[wname] = wt

        # --- bias broadcast tiles (built only when biases are nonzero) ---
        bcast = {}
        if has_bias:
            ones = sml.tile([1, P], f32, tag="ones")
            nc.gpsimd.memset(ones[:], 1.0)
            for bname, bd in (("bc", bc_d), ("bs", bs_d), ("bx", bx_d)):
                brow = sml.tile([1, D], f32, tag="brow")
                nc.sync.dma_start(brow[:], bd[:, :])
                bt = sml.tile([P, D], f32, tag="bb_" + bname)
                pt = ps.tile([P, D], f32, tag="mmout")
                for n0, n1 in NH:
                    nc.tensor.matmul(pt[:, n0:n1], ones[:, :], brow[:, n0:n1],
                                     start=True, stop=True)
                nc.vector.tensor_copy(bt[:], pt[:])
                bcast[bname] = bt

        def stream_T_tiles(src_slab_fn, e, dtype_in):
            """PE-transpose one [128,128] tile -> psum -> evac to f32r sbuf tile."""
            tp = tps.tile([P, P], f32, tag="tp")
            idn = ident if dtype_in == f32 else ident_r
            tpv = tp[:] if dtype_in == f32 else tp[:].bitcast(f32r)
            nc.tensor.matmul(tpv, src_slab_fn(e), idn[:], is_transpose=True,
                             start=True, stop=True)
            ts = p_ts.tile([P, P], f32r, tag="ts")
            nc.any.tensor_copy(ts[:], tp[:])
            return ts

        def mm_statT(src_tile_fn, mov, dtype_in=f32r):
            """For each m: psum[m] = sum_e T(src[m,e]) ... i.e. OUT = SRC @ MOV
            where SRC tiles come in natural orientation and are PE-transposed
            on the fly.  Yields (m, psum_tile)."""
            for m in range(NT):
                pt = ps.tile([P, D], f32, tag="mmout")
                tss = [stream_T_tiles(lambda ee: src_tile_fn(m, ee), e, dtype_in)
                       for e in range(NT)]
                for e in range(NT):
                    for n0, n1 in NH:
                        nc.tensor.matmul(pt[:, n0:n1], tss[e][:], mov[:, e, n0:n1],
                                         start=(e == 0), stop=(e == NT - 1))
                yield m, pt

        def mm_stat(stat, mov):
            """OUT = stat^T-object chain: psum[m] = sum_e stat[e,m]^T @ mov[e]."""
            for m in range(NT):
                pt = ps.tile([P, D], f32, tag="mmout")
                for e in range(NT):
                    for n0, n1 in NH:
                        nc.tensor.matmul(pt[:, n0:n1],
                                         stat[:, e, m * P:(m + 1) * P],
                                         mov[:, e, n0:n1],
                                         start=(e == 0), stop=(e == NT - 1))
                yield m, pt

        def evac(dst, m, pt, add=None, bias=None):
            """PSUM -> SBUF slab copy (rounds to dst dtype); optional residual add."""
            if add is not None:
                nc.vector.tensor_add(dst[:, m], pt[:], add[:, m])
            elif bias is not None:
                nc.vector.tensor_add(dst[:, m], pt[:], bias[:])
            else:
                nc.vector.tensor_copy(dst[:, m], pt[:])
            if add is not None and bias is not None:
                nc.vector.tensor_add(dst[:, m], dst[:, m], bias[:])

        def mat_T(dst, src):
            """dst = transpose(src) materialized, both [P,NT,D] f32r."""
            for j in range(NT):
                pt = ps.tile([P, D], f32, tag="mmout")
                for i in range(NT):
                    nc.tensor.matmul(pt[:, i * P:(i + 1) * P].bitcast(f32r),
                                     src[:, i, j * P:(j + 1) * P], ident_r[:],
                                     is_transpose=True, start=True, stop=True)
                nc.vector.tensor_copy(dst[:, j], pt[:])

        def softmax(dst, m, pt):
            """dst[:,m] = softmax over free dim of psum scores (no max-sub;
            scores are O(30) and exp is safe in fp32)."""
            sums = p_tiny.tile([P, 1], f32, tag="sums")
            nc.scalar.activation(dst[:, m], pt[:], Exp, accum_out=sums[:])
            rec = p_tiny.tile([P, 1], f32, tag="rec")
            nc.vector.reciprocal(rec[:], sums[:])
            nc.scalar.activation(dst[:, m], dst[:, m], Copy, scale=rec[:, 0:1])

        from contextlib import nullcontext
        loop_cm = tc.For_i(0, hw_loop, 1) if hw_loop else nullcontext()
        with loop_cm:
         for _r in range(repeat):
          for b in range(nb):
              # tags pair roles with disjoint lifetimes (see design notes)
              t_lo = sb.tile([P, NT, D], f32r, tag="T1")
              t_io = sb.tile([P, NT, D], f32r, tag="T2")

              # s1/s2: lo = L @ WcT (+bc), io = I @ WcT (+bc).
              # Host supplies L/I pre-transposed as [m][e-part, 128] stationary tiles.
              for (src_d, dst) in ((L_d, t_lo), (I_d, t_io)):
                  for m in range(NT):
                      ltt = p_ltt.tile([P, NT, P], f32r, tag="ltt")
                      for e in range(NT):
                          nc.sync.dma_start(ltt[:, e], src_d[b, m, e * P:(e + 1) * P, :])
                      pt = ps.tile([P, D], f32, tag="mmout")
                      for e in range(NT):
                          for n0, n1 in NH:
                              nc.tensor.matmul(pt[:, n0:n1], ltt[:, e],
                                               w_sb["wc"][:, e, n0:n1],
                                               start=(e == 0), stop=(e == NT - 1))
                      evac(dst, m, pt, bias=bcast.get("bc"))

              # s3: G2 = io^T materialized
              t_G2 = sb.tile([P, NT, D], f32r, tag="T7")
              mat_T(t_G2, t_io)

              # s4: S1 = lo^T... scores = matmul(stat=lo, mov=G2); A1 = softmax
              t_A1 = sb.tile([P, NT, D], f32r, tag="T3")
              for m, pt in mm_stat(t_lo, t_G2):
                  softmax(t_A1, m, pt)

              # s5: co1 = io^T @ A1 = G2 @ A1  (stat=io, mov=A1)
              t_co1 = sb.tile([P, NT, D], f32r, tag="T4")
              for m, pt in mm_stat(t_io, t_A1):
                  evac(t_co1, m, pt)

              # s6: co = co1^T @ WcT + lo^T (+bc): matmuls then lo-transposes into psum
              t_co = sb.tile([P, NT, D], f32r, tag="T5")
              for m in range(NT):
                  pt = ps.tile([P, D], f32, tag="mmout")
                  for e in range(NT):
                      for n0, n1 in NH:
                          nc.tensor.matmul(pt[:, n0:n1],
                                           t_co1[:, e, m * P:(m + 1) * P],
                                           w_sb["wc"][:, e, n0:n1],
                                           start=(e == 0), stop=False)
                  for j in range(NT):
                      nc.tensor.matmul(pt[:, j * P:(j + 1) * P].bitcast(f32r),
                                       t_lo[:, j, m * P:(m + 1) * P], ident_r[:],
                                       is_transpose=True, start=False,
                                       stop=(j in (3, NT - 1)))
                  evac(t_co, m, pt, bias=bcast.get("bc"))

              # s7: sp = co @ WsT (+bs)  (stream-T co tiles as stationary)
              t_sp = sb.tile([P, NT, D], f32r, tag="T1")
              for m, pt in mm_statT(
                      lambda mm, ee: t_co[:, mm, ee * P:(ee + 1) * P], w_sb["ws"]):
                  evac(t_sp, m, pt, bias=bcast.get("bs"))

              # s8: spT materialized
              t_spT = sb.tile([P, NT, D], f32r, tag="T2")
              mat_T(t_spT, t_sp)

              # s9: S2 = sp^T... scores = matmul(stat=sp, mov=spT); A2 = softmax
              t_A2 = sb.tile([P, NT, D], f32r, tag="T3")
              for m, pt in mm_stat(t_sp, t_spT):
                  softmax(t_A2, m, pt)

              # s10: sa1 = A2 @ co  (stream-T A2 tiles as stationary, mov=co)
              t_sa1 = sb.tile([P, NT, D], f32r, tag="T4")
              for m, pt in mm_statT(
                      lambda mm, ee: t_A2[:, mm, ee * P:(ee + 1) * P], t_co):
                  evac(t_sa1, m, pt)

              # s11: sa = sa1^T @ WsT + co (+bs)
              t_sa = sb.tile([P, NT, D], f32r, tag="T6")
              for m, pt in mm_stat(t_sa1, w_sb["ws"]):
                  evac(t_sa, m, pt, add=t_co, bias=bcast.get("bs"))

              # s12: xp = sa @ WxT (+bx)
              t_xp = sb.tile([P, NT, D], f32r, tag="T1")
              for m, pt in mm_statT(
                      lambda mm, ee: t_sa[:, mm, ee * P:(ee + 1) * P], w_sb["wx"]):
                  evac(t_xp, m, pt, bias=bcast.get("bx"))

              # s13: S3 = matmul(stat=xp, mov=G2); A3 = softmax
              t_A3 = sb.tile([P, NT, D], f32r, tag="T3")
              for m, pt in mm_stat(t_xp, t_G2):
                  softmax(t_A3, m, pt)

              # s14: xa1 = A3 @ G2
              t_xa1 = sb.tile([P, NT, D], f32r, tag="T4")
              for m, pt in mm_statT(
                      lambda mm, ee: t_A3[:, mm, ee * P:(ee + 1) * P], t_G2):
                  evac(t_xa1, m, pt)

              # s15: out = xa1^T @ WxT + sa (+bx) -> DMA per slab
              for m, pt in mm_stat(t_xa1, w_sb["wx"]):
                  osl = p_sm.tile([P, D], f32, tag="outsl")
                  nc.vector.tensor_add(osl[:], pt[:], t_sa[:, m])
                  if has_bias:
                      nc.vector.tensor_add(osl[:], osl[:], bcast["bx"][:])
                  nc.sync.dma_start(out_d[b, m * P:(m + 1) * P, :], osl[:])

    nc.finalize()
    return nc


def _get_program(nb, has_bias, repeat=1, hw_loop=0):
    key = (nb, has_bias, repeat, hw_loop)
    if key not in _cache:
        _cache[key] = _build(nb, has_bias, repeat, hw_loop)
    return _cache[key]


def _round_f32r(x):
    """RNE to 11 mantissa bits -- bitwise identical to the on-chip f32r round."""
    xb = np.ascontiguousarray(x, dtype=np.float32).view(np.uint32)
    lsb = (xb >> np.uint32(12)) & np.uint32(1)
    r = (xb + np.uint32(0x7FF) + lsb) & np.uint32(0xFFFFF000)
    return r.view(np.float32)


def kernel(language_output, image_output, Wc, bc, Ws, bs, Wx, bx,
           _n_cores=N_CORES, _nb=None, _repeat=1, _hw_loop=0):
    from concourse import bass_utils

    L0 = np.asarray(language_output, dtype=np.float32)
    I0 = np.asarray(image_output, dtype=np.float32)
    # [B, D, D] -> [B, NT, D, P]: tileT[b, m, :, :] = X[b, m*P:(m+1)*P, :].T
    nbat = L0.shape[0]
    L = _round_f32r(np.ascontiguousarray(
        L0.reshape(nbat, NT, P, D).transpose(0, 1, 3, 2)))
    I = _round_f32r(np.ascontiguousarray(
        I0.reshape(nbat, NT, P, D).transpose(0, 1, 3, 2)))
    wct = _round_f32r(np.asarray(Wc, dtype=np.float32).T)
    wst = _round_f32r(np.asarray(Ws, dtype=np.float32).T)
    wxt = _round_f32r(np.asarray(Wx, dtype=np.float32).T)
    bc_ = np.asarray(bc, dtype=np.float32).reshape(1, D)
    bs_ = np.asarray(bs, dtype=np.float32).reshape(1, D)
    bx_ = np.asarray(bx, dtype=np.float32).reshape(1, D)
    has_bias = bool(np.any(bc_) or np.any(bs_) or np.any(bx_))

    batch = nbat
    n_cores = _n_cores
    nb = _nb if _nb is not None else batch // n_cores
    assert nb * n_cores == batch

    nc = _get_program(nb, has_bias, _repeat, _hw_loop)

    in_maps = []
    for c in range(n_cores):
        sl = slice(c * nb, (c + 1) * nb)
        in_maps.append({
            "L": L[sl], "I": I[sl],
            "wct": wct, "wst": wst, "wxt": wxt,
            "bc": bc_, "bs": bs_, "bx": bx_,
        })
    res = bass_utils.run_bass_kernel_spmd(nc, in_maps, list(range(n_cores)))
    out = np.empty((batch, D, D), dtype=np.float32)
    for c in range(n_cores):
        out[c * nb:(c + 1) * nb] = res.results[c]["out"]
    return out



# revision 2
# speedup vs baseline: 1.2837x; 1.2837x over previous
"""Coattention kernel v2: mat_T+mm_stat everywhere (no stream-T), softmax
scale folded into consumer evac for A2/A3, 3-buf PSUM, stage scopes."""
import numpy as np

B = 32
D = 768
P = 128
NT = D // P  # 6
N_CORES = 8
NB = B // N_CORES

_cache = {}


def _build(nb, has_bias, repeat=1, hw_loop=0):
    import concourse.bass as bass
    import concourse.mybir as mybir
    import concourse.tile as tile
    from concourse import bacc
    from concourse.masks import make_identity
    from contextlib import ExitStack

    f32 = mybir.dt.float32
    f32r = mybir.dt.float32r
    Exp = mybir.ActivationFunctionType.Exp
    Copy = mybir.ActivationFunctionType.Copy

    nc = bacc.Bacc("TRN2", target_bir_lowering=False, debug=False)

    # [nb, m, p, e, q]: element = X[b, m*P+q, e*P+p] (one contiguous DMA per
    # [P, NT, P] stationary-tile group)
    L_d = nc.dram_tensor("L", [nb, NT, P, NT, P], f32r, kind="ExternalInput").ap()
    I_d = nc.dram_tensor("I", [nb, NT, P, NT, P], f32r, kind="ExternalInput").ap()
    wct_d = nc.dram_tensor("wct", [D, D], f32r, kind="ExternalInput").ap()
    wst_d = nc.dram_tensor("wst", [D, D], f32r, kind="ExternalInput").ap()
    wxt_d = nc.dram_tensor("wxt", [D, D], f32r, kind="ExternalInput").ap()
    bc_d = nc.dram_tensor("bc", [1, D], f32, kind="ExternalInput").ap()
    bs_d = nc.dram_tensor("bs", [1, D], f32, kind="ExternalInput").ap()
    bx_d = nc.dram_tensor("bx", [1, D], f32, kind="ExternalInput").ap()
    out_d = nc.dram_tensor("out", [nb, D, D], f32, kind="ExternalOutput").ap()

    NH = ((0, 512), (512, 768))  # psum-bank-aligned halves of the free dim

    with tile.TileContext(nc) as tc, ExitStack() as ctx:
        sb = ctx.enter_context(tc.tile_pool(name="sb", bufs=1))
        sml = ctx.enter_context(tc.tile_pool(name="sml", bufs=1))
        p_ltt = ctx.enter_context(tc.tile_pool(name="p_ltt", bufs=4))
        p_sm = ctx.enter_context(tc.tile_pool(name="p_sm", bufs=2))
        p_tiny = ctx.enter_context(tc.tile_pool(name="p_tiny", bufs=16))
        ps = ctx.enter_context(tc.tile_pool(name="ps", bufs=3, space="PSUM"))

        # --- identities ---
        ident = sml.tile([P, P], f32, tag="ident")
        make_identity(nc, ident[:])
        ident_r = sml.tile([P, P], f32r, tag="identr")
        nc.vector.tensor_copy(ident_r[:], ident[:])

        # --- weights: host pre-rounded to f32r; DMA straight in ---
        w_sb = {}
        for wname, wd in (("wc", wct_d), ("ws", wst_d), ("wx", wxt_d)):
            wt = sb.tile([P, NT, D], f32r, tag="w_" + wname)
            for k in range(NT):
                nc.sync.dma_start(wt[:, k], wd[k * P:(k + 1) * P, :])
            w_sb[wname] = wt

        # --- bias broadcast tiles (built only when biases are nonzero) ---
        bcast = {}
        if has_bias:
            ones = sml.tile([1, P], f32, tag="ones")
            nc.gpsimd.memset(ones[:], 1.0)
            for bname, bd in (("bc", bc_d), ("bs", bs_d), ("bx", bx_d)):
                brow = sml.tile([1, D], f32, tag="brow")
                nc.sync.dma_start(brow[:], bd[:, :])
                bt = sml.tile([P, D], f32, tag="bb_" + bname)
                pt = ps.tile([P, D], f32, tag="mmout")
                for n0, n1 in NH:
                    nc.tensor.matmul(pt[:, n0:n1], ones[:, :], brow[:, n0:n1],
                                     start=True, stop=True)
                nc.vector.tensor_copy(bt[:], pt[:])
                bcast[bname] = bt

        def mm_stat(stat, mov):
            """psum[m] = sum_e stat[:,e,m*P:]^T @ mov[e]  (= STAT^T @ MOV)."""
            for m in range(NT):
                pt = ps.tile([P, D], f32, tag="mmout")
                for e in range(NT):
                    for n0, n1 in NH:
                        nc.tensor.matmul(pt[:, n0:n1],
                                         stat[:, e, m * P:(m + 1) * P],
                                         mov[:, e, n0:n1],
                                         start=(e == 0), stop=(e == NT - 1))
                yield m, pt

        HS = D // 2  # split big evacs DVE/ACT to halve latency on PE chains

        def evac(dst, m, pt, add=None, bias=None, scale=None):
            """PSUM -> SBUF slab copy; optional residual add or row scale."""
            if scale is not None:
                nc.scalar.activation(dst[:, m], pt[:], Copy, scale=scale)
            elif add is not None:
                nc.vector.tensor_add(dst[:, m], pt[:], add[:, m])
            elif bias is not None:
                nc.vector.tensor_add(dst[:, m], pt[:], bias[:])
            else:
                nc.vector.tensor_copy(dst[:, m, :HS], pt[:, :HS])
                nc.scalar.activation(dst[:, m, HS:], pt[:, HS:], Copy)
            if add is not None and bias is not None:
                nc.vector.tensor_add(dst[:, m], dst[:, m], bias[:])

        def mat_T(dst, src):
            """dst = transpose(src) materialized, both [P,NT,D] f32r."""
            for j in range(NT):
                pt = ps.tile([P, D], f32, tag="mmout")
                for i in range(NT):
                    nc.tensor.matmul(pt[:, i * P:(i + 1) * P].bitcast(f32r),
                                     src[:, i, j * P:(j + 1) * P], ident_r[:],
                                     is_transpose=True, start=True, stop=True)
                nc.vector.tensor_copy(dst[:, j], pt[:])

        def softmax(dst, m, pt, rec_out=None):
            """dst[:,m] = exp(pt) [optionally normalized in-place];
            if rec_out is given, store 1/rowsum there instead of scaling."""
            sums = p_tiny.tile([P, 1], f32, tag="sums")
            nc.scalar.activation(dst[:, m], pt[:], Exp, accum_out=sums[:])
            if rec_out is not None:
                nc.vector.reciprocal(rec_out[:, m:m + 1], sums[:])
            else:
                rec = p_tiny.tile([P, 1], f32, tag="rec")
                nc.vector.reciprocal(rec[:], sums[:])
                nc.scalar.activation(dst[:, m], dst[:, m], Copy,
                                     scale=rec[:, 0:1])

        from contextlib import nullcontext
        loop_cm = tc.For_i(0, hw_loop, 1) if hw_loop else nullcontext()
        with loop_cm:
         for _r in range(repeat):
          for b in range(nb):
            # Tag plan (sequential lifetimes per tag):
            #  T1: lo -> sp -> A2T -> xp      T2: io -> coT -> spT -> saT
            #  T3: A1 -> E2 -> E3             T4: co1 -> sa1 -> xa1
            #  T5: co -> A3T                  T6: sa          T7: G2
            t_lo = sb.tile([P, NT, D], f32r, tag="T1")
            t_io = sb.tile([P, NT, D], f32r, tag="T2")

            # s1/s2: lo = L @ WcT (+bc), io = I @ WcT (+bc)
            with nc.named_scope("s12_proj"):
                for (src_d, dst) in ((L_d, t_lo), (I_d, t_io)):
                    for m in range(NT):
                        ltt = p_ltt.tile([P, NT, P], f32r, tag="ltt")
                        nc.sync.dma_start(ltt[:, :, :], src_d[b, m])
                        pt = ps.tile([P, D], f32, tag="mmout")
                        for e in range(NT):
                            for n0, n1 in NH:
                                nc.tensor.matmul(pt[:, n0:n1], ltt[:, e],
                                                 w_sb["wc"][:, e, n0:n1],
                                                 start=(e == 0), stop=(e == NT - 1))
                        evac(dst, m, pt, bias=bcast.get("bc"))

            # s3: G2 = io^T
            t_G2 = sb.tile([P, NT, D], f32r, tag="T7")
            with nc.named_scope("s3_G2"):
                mat_T(t_G2, t_io)

            # s4: scores1 = lo^T @ G2; A1 = softmax (full normalize)
            t_A1 = sb.tile([P, NT, D], f32r, tag="T3")
            with nc.named_scope("s4_scores1"):
                for m, pt in mm_stat(t_lo, t_G2):
                    softmax(t_A1, m, pt)

            # s5: co1 = io^T @ A1
            t_co1 = sb.tile([P, NT, D], f32r, tag="T4")
            with nc.named_scope("s5_co1"):
                for m, pt in mm_stat(t_io, t_A1):
                    evac(t_co1, m, pt)

            # s6: co = co1^T @ WcT + lo^T (+bc)
            t_co = sb.tile([P, NT, D], f32r, tag="T5")
            with nc.named_scope("s6_co"):
                for m in range(NT):
                    pt = ps.tile([P, D], f32, tag="mmout")
                    for e in range(NT):
                        for n0, n1 in NH:
                            nc.tensor.matmul(pt[:, n0:n1],
                                             t_co1[:, e, m * P:(m + 1) * P],
                                             w_sb["wc"][:, e, n0:n1],
                                             start=(e == 0), stop=False)
                    for j in range(NT):
                        nc.tensor.matmul(pt[:, j * P:(j + 1) * P].bitcast(f32r),
                                         t_lo[:, j, m * P:(m + 1) * P], ident_r[:],
                                         is_transpose=True, start=False,
                                         stop=(j in (3, NT - 1)))
                    evac(t_co, m, pt, bias=bcast.get("bc"))

            # s7: sp = co @ WsT (+bs) = mm_stat(coT, ws)
            t_coT = sb.tile([P, NT, D], f32r, tag="T2")
            with nc.named_scope("s7_spT"):
                mat_T(t_coT, t_co)
            t_sp = sb.tile([P, NT, D], f32r, tag="T1")
            with nc.named_scope("s7_sp"):
                for m, pt in mm_stat(t_coT, w_sb["ws"]):
                    evac(t_sp, m, pt, bias=bcast.get("bs"))

            # s8: spT materialized
            t_spT = sb.tile([P, NT, D], f32r, tag="T2")
            with nc.named_scope("s8_spT"):
                mat_T(t_spT, t_sp)

            # s9: scores2 = sp^T spT; E2 = exp (unnormalized), r2 = 1/rowsum
            t_A2 = sb.tile([P, NT, D], f32r, tag="T3")
            r2 = p_tiny.tile([P, NT], f32, tag="r2")
            with nc.named_scope("s9_scores2"):
                for m, pt in mm_stat(t_sp, t_spT):
                    softmax(t_A2, m, pt, rec_out=r2)

            # s10: sa1 = diag(r2) E2 co = mm_stat(E2T, co) scaled at evac
            t_A2T = sb.tile([P, NT, D], f32r, tag="T1")
            with nc.named_scope("s10_A2T"):
                mat_T(t_A2T, t_A2)
            t_sa1 = sb.tile([P, NT, D], f32r, tag="T4")
            with nc.named_scope("s10_sa1"):
                for m, pt in mm_stat(t_A2T, t_co):
                    evac(t_sa1, m, pt, scale=r2[:, m:m + 1])

            # s11: sa = sa1^T @ WsT + co (+bs)
            t_sa = sb.tile([P, NT, D], f32r, tag="T6")
            with nc.named_scope("s11_sa"):
                for m, pt in mm_stat(t_sa1, w_sb["ws"]):
                    evac(t_sa, m, pt, add=t_co, bias=bcast.get("bs"))

            # s12: xp = sa @ WxT (+bx) = mm_stat(saT, wx)
            t_saT = sb.tile([P, NT, D], f32r, tag="T2")
            with nc.named_scope("s12_saT"):
                mat_T(t_saT, t_sa)
            t_xp = sb.tile([P, NT, D], f32r, tag="T1")
            with nc.named_scope("s12_xp"):
                for m, pt in mm_stat(t_saT, w_sb["wx"]):
                    evac(t_xp, m, pt, bias=bcast.get("bx"))

            # s13: scores3 = xp^T G2; E3 = exp, r3 = 1/rowsum
            t_A3 = sb.tile([P, NT, D], f32r, tag="T3")
            r3 = p_tiny.tile([P, NT], f32, tag="r3")
            with nc.named_scope("s13_scores3"):
                for m, pt in mm_stat(t_xp, t_G2):
                    softmax(t_A3, m, pt, rec_out=r3)

            # s14: xa1 = diag(r3) E3 G2 = mm_stat(E3T, G2) scaled at evac
            t_A3T = sb.tile([P, NT, D], f32r, tag="T5")
            with nc.named_scope("s14_A3T"):
                mat_T(t_A3T, t_A3)
            t_xa1 = sb.tile([P, NT, D], f32r, tag="T4")
            with nc.named_scope("s14_xa1"):
                for m, pt in mm_stat(t_A3T, t_G2):
                    evac(t_xa1, m, pt, scale=r3[:, m:m + 1])

            # s15: out = xa1^T @ WxT + sa (+bx) -> DMA per slab
            with nc.named_scope("s15_out"):
                for m, pt in mm_stat(t_xa1, w_sb["wx"]):
                    osl = p_sm.tile([P, D], f32, tag="outsl")
                    nc.vector.tensor_add(osl[:], pt[:], t_sa[:, m])
                    if has_bias:
                        nc.vector.tensor_add(osl[:], osl[:], bcast["bx"][:])
                    # qAct HWDGE ring: keep output stores off the qSP ring so
                    # next-batch input loads aren't head-of-line blocked.
                    nc.scalar.dma_start(out_d[b, m * P:(m + 1) * P, :], osl[:])

    nc.finalize()
    return nc


def _get_program(nb, has_bias, repeat=1, hw_loop=0):
    key = (nb, has_bias, repeat, hw_loop)
    if key not in _cache:
        _cache[key] = _build(nb, has_bias, repeat, hw_loop)
    return _cache[key]


def _round_f32r(x):
    """RNE to 11 mantissa bits -- bitwise identical to the on-chip f32r round."""
    xb = np.ascontiguousarray(x, dtype=np.float32).view(np.uint32)
    lsb = (xb >> np.uint32(12)) & np.uint32(1)
    r = (xb + np.uint32(0x7FF) + lsb) & np.uint32(0xFFFFF000)
    return r.view(np.float32)


def kernel(language_output, image_output, Wc, bc, Ws, bs, Wx, bx,
           _n_cores=N_CORES, _nb=None, _repeat=1, _hw_loop=0):
    from concourse import bass_utils

    L0 = np.asarray(language_output, dtype=np.float32)
    I0 = np.asarray(image_output, dtype=np.float32)
    nbat = L0.shape[0]
    # [b, m, p, e, q] = X[b, m*P+q, e*P+p]
    L = _round_f32r(np.ascontiguousarray(
        L0.reshape(nbat, NT, P, NT, P).transpose(0, 1, 4, 3, 2)))
    I = _round_f32r(np.ascontiguousarray(
        I0.reshape(nbat, NT, P, NT, P).transpose(0, 1, 4, 3, 2)))
    wct = _round_f32r(np.asarray(Wc, dtype=np.float32).T)
    wst = _round_f32r(np.asarray(Ws, dtype=np.float32).T)
    wxt = _round_f32r(np.asarray(Wx, dtype=np.float32).T)
    bc_ = np.asarray(bc, dtype=np.float32).reshape(1, D)
    bs_ = np.asarray(bs, dtype=np.float32).reshape(1, D)
    bx_ = np.asarray(bx, dtype=np.float32).reshape(1, D)
    has_bias = bool(np.any(bc_) or np.any(bs_) or np.any(bx_))

    batch = nbat
    n_cores = _n_cores
    nb = _nb if _nb is not None else batch // n_cores
    assert nb * n_cores == batch

    nc = _get_program(nb, has_bias, _repeat, _hw_loop)

    in_maps = []
    for c in range(n_cores):
        sl = slice(c * nb, (c + 1) * nb)
        in_maps.append({
            "L": L[sl], "I": I[sl],
            "wct": wct, "wst": wst, "wxt": wxt,
            "bc": bc_, "bs": bs_, "bx": bx_,
        })
    res = bass_utils.run_bass_kernel_spmd(nc, in_maps, list(range(n_cores)))
    out = np.empty((batch, D, D), dtype=np.float32)
    for c in range(n_cores):
        out[c * nb:(c + 1) * nb] = res.results[c]["out"]
    return out
